# revision 42
# baseline (speedup 1.0000x reference)
"""GQA attention (B=2, S=2048, 16 Q heads / 8 KV heads, head_dim=128, RoPE,
no causal mask) on 8 Trainium2 NeuronCores.

Sharding: DP=2 on batch x TP=4 on heads. Each core computes 4 Q heads /
2 KV heads for one batch element, plus a row-sharded o_proj partial; the
host sums the 4 partials per batch (the "all-reduce").

v2 design (from trace analysis of the 485us baseline):
 - all matmul operands bf16 (1 cycle/row, same PE rate as f32r, half the
   DMA/SBUF) with f32 PSUM accumulation
 - Q/K/V/oh fully SBUF-resident: no DRAM roundtrip between phases
 - softmax denominator via DVE pairwise-tree sum + ONE ones-matmul per
   unit (was 16 matmuls/unit = 58us of PE time)
 - reciprocal_approx_fast on a [1,512] row + gpsimd partition_broadcast
   (was vector.reciprocal on [128,512] = 3.4us/unit)
 - PSUM evacuations on DVE, keeping the Scalar engine exclusively for
   exp (ACT is the phase-B critical engine at ~11us/unit)
 - software pipeline: PV + denominator of unit u-1 emitted between the
   score matmuls of unit u; o_proj tiles spread 4-per-unit
 - DMA issue order = first-use order (wk, x q0, wv, cs, wq, ...); x
   double-buffered in 512-column quarters
"""
import json
import math
from contextlib import ExitStack

import numpy as np

# ---------------------------------------------------------------------------
# Environment patches (required for the walrus build in this container)
# ---------------------------------------------------------------------------
_PATCHED = False


def _install_patches():
    """1) The walrus here rejects >1 sync wait per instruction; split extra
    waits onto single-wait NoOps inserted before the instruction (engines
    execute their stream in order, so semantics are preserved).
    2) antenv.axon_hooks is missing in this image; shim it so trace=True
    profiling works (used by test harnesses; harmless otherwise)."""
    global _PATCHED
    if _PATCHED:
        return
    _PATCHED = True

    import concourse.bass as bass

    counter = [0]

    def _split_multiwait(bir):
        for func in bir.get("functions", []):
            for block in func.get("blocks", []):
                new_insts = []
                for inst in block.get("instructions", []):
                    si = inst.get("sync_info")
                    waits = (si or {}).get("on_wait") or []
                    if len(waits) > 1:
                        for w in waits[:-1]:
                            counter[0] += 1
                            new_insts.append(
                                {
                                    "debug": inst.get("debug", 0),
                                    "engine": inst.get("engine"),
                                    "ins": [],
                                    "name": f"I-waitsplit-{counter[0]}",
                                    "opcode": "NoOp",
                                    "outs": [],
                                    "sync_info": {"on_wait": [w], "on_update": []},
                                }
                            )
                        si["on_wait"] = [waits[-1]]
                    new_insts.append(inst)
                block["instructions"] = new_insts
        return bir

    orig_to_json_bytes = bass.Bass.to_json_bytes

    def patched_to_json_bytes(self):
        bir = json.loads(orig_to_json_bytes(self))
        return json.dumps(_split_multiwait(bir)).encode()

    bass.Bass.to_json_bytes = patched_to_json_bytes

    # -- NTFF profile hook shim (for trace=True) --
    import sys
    import types

    if "antenv.axon_hooks" not in sys.modules:
        mod = types.ModuleType("antenv.axon_hooks")
        _hook = [None]
        try:
            from trn_agent_boot.trn_boot import _ntff_profile_via_ctypes

            _hook[0] = _ntff_profile_via_ctypes("/opt/axon/libaxon_pjrt.so")
        except Exception:
            pass
        mod.get_axon_ntff_profile_hook = lambda: _hook[0]
        mod.set_axon_ntff_profile_hook = lambda h: _hook.__setitem__(0, h)
        sys.modules["antenv.axon_hooks"] = mod

    # upload_artifacts needs external storage; make it a no-op locally.
    import concourse.bass_utils as bu

    bu.upload_artifacts = lambda tmpdir: str(tmpdir)


# ---------------------------------------------------------------------------
# Problem constants (hardcoded per contest contract)
# ---------------------------------------------------------------------------
B, S, HID = 2, 2048, 2048
N_HEADS, N_KV = 16, 8
HD = 128
TP = 4  # tensor-parallel factor over heads
NQ = N_HEADS // TP  # 4 q heads per core
NKV = N_KV // TP  # 2 kv heads per core
KT = HID // 128  # 16 contraction tiles
ST = S // 128  # 16 sequence tiles of 128
SC = 512  # free-dim chunk
NB = S // SC  # 4 chunks over S
SCALE = 1.0 / math.sqrt(HD)


def _build_nc():
    import concourse.bass as bass
    import concourse.tile as tile
    from concourse import bass_isa, mybir

    f32 = mybir.dt.float32
    bf16 = mybir.dt.bfloat16
    AF = mybir.ActivationFunctionType

    nc = bass.Bass()
    xT = nc.dram_tensor("xT", [HID, S], bf16, kind="ExternalInput")
    wq = nc.dram_tensor("wq", [HID, NQ * HD], bf16, kind="ExternalInput")
    wk = nc.dram_tensor("wk", [HID, NKV * HD], bf16, kind="ExternalInput")
    wv = nc.dram_tensor("wv", [HID, NKV * HD], bf16, kind="ExternalInput")
    wo = nc.dram_tensor("wo", [NQ * HD, HID], bf16, kind="ExternalInput")
    # stacked trig: cs1 = [cos; sin], cs2 = [sin; cos] on the partition axis
    # (keeps every SBUF operand pair of the RoPE muls at the same base
    # partition — a walrus requirement)
    cs1d = nc.dram_tensor("cs1", [HD, S], bf16, kind="ExternalInput")
    cs2d = nc.dram_tensor("cs2", [HD, S], bf16, kind="ExternalInput")
    # bf16 partials: host upcasts and sums; halves the output DMA drain
    out = nc.dram_tensor("out", [S, HID], bf16, kind="ExternalOutput")

    with tile.TileContext(nc) as tc, ExitStack() as ctx:
        const = ctx.enter_context(tc.tile_pool(name="const", bufs=1))
        wpool = ctx.enter_context(tc.tile_pool(name="wpool", bufs=1))
        res = ctx.enter_context(tc.tile_pool(name="res", bufs=1))

        ones_bf = const.tile([128, 128], bf16, tag="ones")
        nc.vector.memset(ones_bf[:], 1.0)

        # resident weights / trig
        wk_sb = wpool.tile([128, KT, NKV * HD], bf16, tag="wk")
        wv_sb = wpool.tile([128, KT, NKV * HD], bf16, tag="wv")
        wq_sb = wpool.tile([128, KT, NQ * HD], bf16, tag="wq")
        wo_sb = wpool.tile([128, NQ, HID], bf16, tag="wo")
        cs1 = wpool.tile([HD, S], bf16, tag="cs1")
        cs2 = wpool.tile([HD, S], bf16, tag="cs2")

        # resident activations
        kT_sb = [res.tile([128, S], bf16, tag=f"kT{g}", name=f"kT{g}") for g in range(NKV)]
        q_sb = res.tile([128, NQ, S], bf16, tag="q", name="q")
        v_sb = res.tile([128, ST, NKV * HD], bf16, tag="v", name="v")
        oh_t = [res.tile([128, S], bf16, tag=f"oh{h}", name=f"oh{h}") for h in range(NQ)]

        # ------------------------- Phase A: projections + RoPE ----------
        def rope(ps, c0, sink):
            """ps: [128,SC] f32 PSUM, rows 0:64 = re, 64:128 = im.
            sink: [128,SC] bf16 slice. out_re = re*cos - im*sin,
            out_im = re*sin + im*cos.  The ACT engine (idle in phase A)
            evacuates PSUM to bf16; the muls/sub/add then run all-bf16
            all-SBUF packed = 4x DVE mode.  Stacked trig tiles keep each
            SBUF operand pair at the same base partition (walrus rule):
            t2 pairs pb[64:] with cs1[64:]=sin, t4 pairs pb[64:] with
            cs2[64:]=cos."""
            pb = ropest.tile([128, SC], bf16, tag="pb")
            nc.scalar.copy(pb[:], ps[:])
            t1 = ropest.tile([64, SC], bf16, tag="t1")
            t2 = ropest.tile([64, SC], bf16, tag="t2")
            t3 = ropest.tile([64, SC], bf16, tag="t3")
            t4 = ropest.tile([64, SC], bf16, tag="t4")
            nc.vector.tensor_mul(t1[:], pb[0:64, :], cs1[0:64, c0 : c0 + SC])
            nc.vector.tensor_mul(t2[:], pb[64:128, :], cs1[64:128, c0 : c0 + SC])
            nc.vector.tensor_mul(t3[:], pb[0:64, :], cs2[0:64, c0 : c0 + SC])
            nc.vector.tensor_mul(t4[:], pb[64:128, :], cs2[64:128, c0 : c0 + SC])
            nc.vector.tensor_sub(sink[0:64, :], t1[:], t2[:])
            nc.vector.tensor_add(sink[64:128, :], t3[:], t4[:])

        with ExitStack() as actx:
            xpool = actx.enter_context(tc.tile_pool(name="xpool", bufs=4))
            ropest = actx.enter_context(tc.tile_pool(name="ropest", bufs=2))
            pmm = actx.enter_context(
                tc.tile_pool(name="pmm", bufs=3, space="PSUM")
            )
            pvv = actx.enter_context(
                tc.tile_pool(name="pvv", bufs=2, space="PSUM")
            )

            # All 4 x-quarters stay resident (xpool bufs=4); phase A runs
            # K-all -> V-all -> Q-all so each weight tensor arrives well
            # before its consumer loop (wv by ~33us, wq by ~62us) instead
            # of stalling quarter 0.
            wk_re = wk.rearrange("(kt p) d -> p kt d", p=128)
            # head of the K-g0 weight column + first x tiles go first in
            # ~128KB chunks so the first matmul chain starts ~5us earlier
            nc.sync.dma_start(out=wk_sb[:, 0:4, 0:HD], in_=wk_re[:, 0:4, 0:HD])
            x_tiles = []
            xt0 = xpool.tile([128, KT, SC], bf16, tag="x", name="x0")
            for kt in range(2):
                nc.sync.dma_start(out=xt0[:, kt, :], in_=xT[kt * 128 : (kt + 1) * 128, 0:SC])
            nc.sync.dma_start(out=wk_sb[:, 4:, 0:HD], in_=wk_re[:, 4:, 0:HD])
            for kt in range(2, 4):
                nc.sync.dma_start(out=xt0[:, kt, :], in_=xT[kt * 128 : (kt + 1) * 128, 0:SC])
            nc.sync.dma_start(out=wk_sb[:, :, HD:], in_=wk_re[:, :, HD:])
            for kt in range(4, KT):
                nc.sync.dma_start(out=xt0[:, kt, :], in_=xT[kt * 128 : (kt + 1) * 128, 0:SC])
            x_tiles.append(xt0)
            nc.sync.dma_start(out=cs1[:], in_=cs1d[:, :])
            nc.sync.dma_start(out=cs2[:], in_=cs2d[:, :])
            for nb in range(1, NB):
                xt = xpool.tile([128, KT, SC], bf16, tag="x", name=f"x{nb}")
                for kt in range(KT):
                    nc.sync.dma_start(
                        out=xt[:, kt, :],
                        in_=xT[kt * 128 : (kt + 1) * 128, nb * SC : (nb + 1) * SC],
                    )
                x_tiles.append(xt)
                if nb == 1:
                    nc.sync.dma_start(
                        out=wv_sb[:],
                        in_=wv.rearrange("(kt p) d -> p kt d", p=128),
                    )
                if nb == 2:
                    nc.sync.dma_start(
                        out=wq_sb[:],
                        in_=wq.rearrange("(kt p) d -> p kt d", p=128),
                    )
            nc.sync.dma_start(
                out=wo_sb[:], in_=wo.rearrange("(dt p) n -> p dt n", p=128)
            )

            # K projection + RoPE, all quarters
            for nb in range(NB):
                c0 = nb * SC
                for g in range(NKV):
                    ps = pmm.tile([128, SC], f32, tag="mm")
                    for kt in range(KT):
                        nc.tensor.matmul(
                            ps[:],
                            wk_sb[:, kt, g * HD : (g + 1) * HD],
                            x_tiles[nb][:, kt, :],
                            start=(kt == 0),
                            stop=(kt == KT - 1),
                        )
                    rope(ps, c0, kT_sb[g][:, c0 : c0 + SC])

            # V projection, all quarters
            for nb in range(NB):
                for st in range(4):
                    ps = pvv.tile([128, NKV * HD], f32, tag="vv")
                    for kt in range(KT):
                        nc.tensor.matmul(
                            ps[:],
                            x_tiles[nb][:, kt, st * 128 : (st + 1) * 128],
                            wv_sb[:, kt, :],
                            start=(kt == 0),
                            stop=(kt == KT - 1),
                        )
                    nc.scalar.copy(v_sb[:, nb * 4 + st, :], ps[:])

            # Q projection + RoPE, all quarters
            for nb in range(NB):
                c0 = nb * SC
                for h in range(NQ):
                    ps = pmm.tile([128, SC], f32, tag="mm")
                    for kt in range(KT):
                        nc.tensor.matmul(
                            ps[:],
                            wq_sb[:, kt, h * HD : (h + 1) * HD],
                            x_tiles[nb][:, kt, :],
                            start=(kt == 0),
                            stop=(kt == KT - 1),
                        )
                    rope(ps, c0, q_sb[:, h, c0 : c0 + SC])

        # ------------- Phase B: attention + interleaved o_proj ----------
        with ExitStack() as bctx:
            ptpool = bctx.enter_context(tc.tile_pool(name="ptpool", bufs=2))
            dtree = bctx.enter_context(tc.tile_pool(name="dtree", bufs=1))
            s1pool = bctx.enter_context(tc.tile_pool(name="s1pool", bufs=2))
            recpool = bctx.enter_context(tc.tile_pool(name="recpool", bufs=2))
            ostage = bctx.enter_context(tc.tile_pool(name="ostage", bufs=3))
            spool = bctx.enter_context(
                tc.tile_pool(name="spool", bufs=3, space="PSUM")
            )
            pvpool = bctx.enter_context(
                tc.tile_pool(name="pvpool", bufs=2, space="PSUM")
            )
            denpool = bctx.enter_context(
                tc.tile_pool(name="denpool", bufs=1, space="PSUM")
            )
            oppool = bctx.enter_context(
                tc.tile_pool(name="oppool", bufs=2, space="PSUM")
            )

            units = [(nb, h) for nb in range(NB) for h in range(NQ)]

            def emit_denom_tree(pt):
                """bf16 pairwise tree over kt -> s1 [128,SC]: 4 wide DVE
                ops (packed bf16 all-SBUF = 4x mode).  pt is laid out
                [128, KT//2, 2, SC]; each level's pair axis is the
                trailing '2' of the previous tile (free-size-only shape
                check makes the dim mismatch legal)."""
                t8 = dtree.tile([128, 4, 2, SC], bf16, tag="t8")
                nc.vector.tensor_add(t8[:], pt[:, :, 0, :], pt[:, :, 1, :])
                t4 = dtree.tile([128, 2, 2, SC], bf16, tag="t4")
                nc.vector.tensor_add(t4[:], t8[:, :, 0, :], t8[:, :, 1, :])
                t2 = dtree.tile([128, 2, SC], bf16, tag="t2")
                nc.vector.tensor_add(t2[:], t4[:, :, 0, :], t4[:, :, 1, :])
                s1 = s1pool.tile([128, SC], bf16, tag="s1")
                nc.vector.tensor_add(s1[:], t2[:, 0, :], t2[:, 1, :])
                return s1

            def emit_den_rec(s1):
                """ones-matmul broadcasts den over partitions; approx
                reciprocal (free-size-costed, ~0.7us) inverts the full
                tile."""
                psden = denpool.tile([128, SC], f32, tag="den")
                nc.tensor.matmul(
                    psden[:], ones_bf[:], s1[:], start=True, stop=True
                )
                recb = recpool.tile([128, SC], f32, tag="recb")
                nc.vector.reciprocal(recb[:], psden[:])
                return recb

            # o_proj tile state machine: one matmul per slot
            oproj_queue = []  # (st, nn) tiles ready to compute
            op_state = [None, 0]  # [(st, nn, pso), next_h]

            def emit_oproj_mm():
                if op_state[0] is None:
                    if not oproj_queue:
                        return
                    st, nn = oproj_queue.pop(0)
                    op_state[0] = (
                        st,
                        nn,
                        oppool.tile([128, SC], f32, tag="op", name="pso"),
                    )
                    op_state[1] = 0
                st, nn, pso = op_state[0]
                h = op_state[1]
                nc.tensor.matmul(
                    pso[:],
                    oh_t[h][:, st * 128 : (st + 1) * 128],
                    wo_sb[:, h, nn * SC : (nn + 1) * SC],
                    start=(h == 0),
                    stop=(h == NQ - 1),
                )
                op_state[1] += 1
                if op_state[1] == NQ:
                    ot = ostage.tile([128, SC], bf16, tag="ot")
                    nc.vector.tensor_copy(ot[:], pso[:])
                    nc.sync.dma_start(
                        out=out[
                            st * 128 : (st + 1) * 128, nn * SC : (nn + 1) * SC
                        ],
                        in_=ot[:],
                    )
                    op_state[0] = None

            # Steady state per iteration u (slot kt): one score-mm (feeds
            # ACT's exp, the pacing engine), one PV-mm of unit u-1, one
            # o_proj-mm.  Denominator chain of u-1 rides at slots 0/4.
            prev = None
            for u, (nb, h) in enumerate(units):
                g = h // (NQ // NKV)
                pt = ptpool.tile([128, KT // 2, 2, SC], bf16, tag="pt")
                ppv_prev = None
                if prev is not None:
                    ppv_prev = pvpool.tile([128, SC], f32, tag="pv")
                for kt in range(KT):
                    pss = spool.tile([128, SC], f32, tag="ss")
                    nc.tensor.matmul(
                        pss[:],
                        kT_sb[g][:, kt * 128 : (kt + 1) * 128],
                        q_sb[:, h, nb * SC : (nb + 1) * SC],
                        start=True,
                        stop=True,
                    )
                    nc.scalar.activation(
                        pt[:, kt // 2, kt % 2, :], pss[:], AF.Exp, scale=SCALE
                    )
                    if prev is not None:
                        pnb, ph, pg, ppt = prev
                        if kt == 0:
                            ptree = emit_denom_tree(ppt)
                        nc.tensor.matmul(
                            ppv_prev[:],
                            v_sb[:, kt, pg * HD : (pg + 1) * HD],
                            ppt[:, kt // 2, kt % 2, :],
                            start=(kt == 0),
                            stop=(kt == KT - 1),
                        )
                        if kt == 7:
                            # late enough that the DVE tree is done even
                            # in early iterations with no o_proj fill —
                            # the PE otherwise stalls here and starves
                            # the exp stream
                            precb = emit_den_rec(ptree)
                    emit_oproj_mm()
                    if u >= 13 and kt % 8 == 0:
                        # drain the o_proj backlog faster near the end to
                        # shrink the post-exp epilogue
                        emit_oproj_mm()
                if prev is not None:
                    pnb, ph, pg, ppt = prev
                    nc.vector.tensor_mul(
                        oh_t[ph][:, pnb * SC : (pnb + 1) * SC],
                        ppv_prev[:],
                        precb[:],
                    )
                prev = (nb, h, g, pt)
                # round r's oh tiles are fully written (in program order)
                # only after the oh-mul emitted at the END of iteration
                # u = 4r+4 — so queue round r-1's o_proj tiles here.
                if u % NQ == 0 and u > 0:
                    r = u // NQ - 1
                    oproj_queue.extend(
                        (r * 4 + sti, nn) for sti in range(4) for nn in range(NB)
                    )
            # epilogue: PV + denominator of the last unit, then remaining
            # o_proj tiles
            pnb, ph, pg, ppt = prev
            ptree = emit_denom_tree(ppt)
            precb = emit_den_rec(ptree)
            ppv_last = pvpool.tile([128, SC], f32, tag="pv")
            for kt in range(KT):
                nc.tensor.matmul(
                    ppv_last[:],
                    v_sb[:, kt, pg * HD : (pg + 1) * HD],
                    ppt[:, kt // 2, kt % 2, :],
                    start=(kt == 0),
                    stop=(kt == KT - 1),
                )
                emit_oproj_mm()
            nc.vector.tensor_mul(
                oh_t[ph][:, pnb * SC : (pnb + 1) * SC], ppv_last[:], precb[:]
            )
            oproj_queue.extend(
                (12 + sti, nn) for sti in range(4) for nn in range(NB)
            )
            while oproj_queue or op_state[0] is not None:
                emit_oproj_mm()
    return nc


_NC_CACHE = None


def _get_nc():
    global _NC_CACHE
    if _NC_CACHE is None:
        _install_patches()
        _NC_CACHE = _build_nc()
    return _NC_CACHE


# De-interleave permutation: within each head, even dims then odd dims.
_PERM = np.concatenate([np.arange(0, HD, 2), np.arange(1, HD, 2)])

_last_in_maps = None


def kernel(x, Wq, Wk, Wv, Wo, freqs_cos, freqs_sin, start_pos):
    _install_patches()
    import ml_dtypes

    from concourse.bass_utils import run_bass_kernel_spmd

    bf16 = ml_dtypes.bfloat16

    x = np.asarray(x, dtype=np.float32)
    Wq = np.asarray(Wq, dtype=np.float32)
    Wk = np.asarray(Wk, dtype=np.float32)
    Wv = np.asarray(Wv, dtype=np.float32)
    Wo = np.asarray(Wo, dtype=np.float32)
    cosT = np.asarray(freqs_cos, dtype=np.float32).T  # [64, S]
    sinT = np.asarray(freqs_sin, dtype=np.float32).T
    cs1 = np.ascontiguousarray(np.concatenate([cosT, sinT], 0)).astype(bf16)
    cs2 = np.ascontiguousarray(np.concatenate([sinT, cosT], 0)).astype(bf16)

    # Per-head de-interleave of Wq/Wk columns (RoPE pairs -> [re, im] blocks)
    Wq_p = Wq.reshape(HID, N_HEADS, HD)[:, :, _PERM]
    Wk_p = Wk.reshape(HID, N_KV, HD)[:, :, _PERM]

    in_maps = []
    for core in range(8):
        b, t = divmod(core, TP)
        in_maps.append(
            {
                "xT": np.ascontiguousarray(x[b].T).astype(bf16),
                "wq": np.ascontiguousarray(
                    Wq_p[:, t * NQ : (t + 1) * NQ, :].reshape(HID, NQ * HD)
                ).astype(bf16),
                "wk": np.ascontiguousarray(
                    Wk_p[:, t * NKV : (t + 1) * NKV, :].reshape(HID, NKV * HD)
                ).astype(bf16),
                "wv": np.ascontiguousarray(
                    Wv.reshape(HID, N_KV, HD)[:, t * NKV : (t + 1) * NKV, :].reshape(
                        HID, NKV * HD
                    )
                ).astype(bf16),
                "wo": np.ascontiguousarray(
                    Wo[t * NQ * HD : (t + 1) * NQ * HD, :]
                ).astype(bf16),
                "cs1": cs1,
                "cs2": cs2,
            }
        )

    global _last_in_maps
    _last_in_maps = in_maps
    nc = _get_nc()
    res = run_bass_kernel_spmd(nc, in_maps, list(range(8)))
    outs = [
        np.asarray(res.results[c]["out"]).astype(np.float32) for c in range(8)
    ]
    full = np.stack(
        [sum(outs[b * TP + t] for t in range(TP)) for b in range(B)]
    ).astype(np.float32)
    return full
